# revision 5
# baseline (speedup 1.0000x reference)
"""MetaPathAgg Trainium2 kernel (8 NeuronCores, SPMD).

Algebraic restructuring: out[e] = LT_table[vote_lt[e]] + BV_table[vote_bv[e]]
where
  LT_table = h_lt @ W0 + mean_mem @ W3 + (mean_don + mean_lob) @ W4 + b_fuse
  BV_table[v] = mean_pv[v] @ W1 + bill_table[bv2bill[v]]
  bill_table[b] = bill_comm[b] @ W2 + (h_topic @ W5)[topic_ix[b]]
  bill_comm[b] = mean over versions v of b of mean_rd[v]

All segment-means run as: indirect-DMA row gather + is_equal one-hot +
PSUM-accumulated matmuls, destination-sharded over 8 cores so no
all-reduce is needed (only a 2.5 MB LT-table AllGather).
"""

import math
import os
import sys

import numpy as np

sys.path.insert(0, "/opt/trn_rl_repo")

import concourse.bass as bass  # noqa: E402
import concourse.bacc as bacc  # noqa: E402
import concourse.mybir as mybir  # noqa: E402
import concourse.tile as tile  # noqa: E402

CORES = 8
P = 128
D = 128
SUP = 16       # gather slots batched per indirect DMA / one-hot op
VCAP = 256     # bill-versions per bill-block (2 windows of 128)
BCAP = 128     # bills per bill-block
NVB = VCAP // P

F32 = mybir.dt.float32
I32 = mybir.dt.int32

_LAST_EXEC_NS = None


def _expand_last(ap, n):
    """[.., k] AP -> [.., k, n] with a step-0 broadcast dim appended."""
    return bass.AP(ap.tensor, ap.offset, list(ap.ap) + [[0, n]])


def _ceil(a, b):
    return (a + b - 1) // b


# ---------------------------------------------------------------------------
# host-side integer preprocessing
# ---------------------------------------------------------------------------

def _prep_segsum(feat_idx, owner, local, nwin):
    """Pack edges of one relation into per-core window-aligned 128-slots.

    Returns idxT [CORES,P,SPAD] int32 (gather row ids; pads point at row 0),
    locT [CORES,P,SPAD] f32 (dst offset within its 128-window; pads -1),
    win_of_slot [SPAD].
    """
    cnt = np.zeros((CORES, nwin), np.int64)
    np.add.at(cnt, (owner, local // P), 1)
    nslot_w = _ceil(cnt, P).max(axis=0)
    slot_base = np.concatenate([[0], np.cumsum(nslot_w)]).astype(np.int64)
    S = int(slot_base[-1])
    SPAD = max(_ceil(S, SUP) * SUP, SUP)
    win_of_slot = np.repeat(np.arange(nwin), nslot_w)
    win_of_slot = np.concatenate(
        [win_of_slot, np.full(SPAD - S, max(nwin - 1, 0))]).astype(np.int64)
    idxT = np.zeros((CORES, P, SPAD), np.int32)
    locT = np.full((CORES, P, SPAD), -1.0, np.float32)
    for c in range(CORES):
        m = owner == c
        fi = feat_idx[m]
        lo = local[m]
        order = np.argsort(lo, kind="stable")
        fi = fi[order]
        lo = lo[order]
        w = lo // P
        wstart = np.searchsorted(w, np.arange(nwin))
        r = np.arange(len(lo)) - wstart[w]
        pos = slot_base[w] * P + r
        slot = pos // P
        part = pos % P
        idxT[c, part, slot] = fi
        locT[c, part, slot] = (lo - w * P).astype(np.float32)
    return dict(idxT=idxT, locT=locT, wos=win_of_slot, S=SPAD)


def _recipT(counts_local, nwin):
    """counts over local dst space -> [P, nwin] per-partition recip slab."""
    r = np.ones(nwin * P, np.float32)
    n = len(counts_local)
    r[:n] = 1.0 / np.maximum(counts_local, 1)
    return r.reshape(nwin, P).T.copy()


def _prep(inputs):
    h_bv = np.asarray(inputs["h_bv"], np.float32)
    h_lt = np.asarray(inputs["h_lt"], np.float32)
    h_comm = np.asarray(inputs["h_comm"], np.float32)
    h_donor = np.asarray(inputs["h_donor"], np.float32)
    h_lobby = np.asarray(inputs["h_lobby"], np.float32)
    h_topic = np.asarray(inputs["h_topic"], np.float32)
    W_fuse = np.asarray(inputs["W_fuse"], np.float32)
    b_fuse = np.asarray(inputs["b_fuse"], np.float32)
    vote_lt = np.asarray(inputs["vote_lt"]).astype(np.int64)
    vote_bv = np.asarray(inputs["vote_bv"]).astype(np.int64)
    bv2bill = np.asarray(inputs["bv2bill"]).astype(np.int64)
    topic_ix = np.asarray(inputs["topic_ix"]).astype(np.int64)
    pv_src = np.asarray(inputs["pv_src"]).astype(np.int64)
    pv_dst = np.asarray(inputs["pv_dst"]).astype(np.int64)
    r_src = np.asarray(inputs["r_src"]).astype(np.int64)
    r_dst = np.asarray(inputs["r_dst"]).astype(np.int64)
    m_src = np.asarray(inputs["m_src"]).astype(np.int64)
    m_dst = np.asarray(inputs["m_dst"]).astype(np.int64)
    don_src = np.asarray(inputs["don_src"]).astype(np.int64)
    don_dst = np.asarray(inputs["don_dst"]).astype(np.int64)
    lob_src = np.asarray(inputs["lob_src"]).astype(np.int64)
    lob_dst = np.asarray(inputs["lob_dst"]).astype(np.int64)

    NBV = h_bv.shape[0]
    NLT = h_lt.shape[0]
    NB = np.asarray(inputs["h_bill"]).shape[0]
    NT = h_topic.shape[0]
    E = vote_lt.shape[0]
    assert NT <= P

    # ---- LT sharding -----------------------------------------------------
    LTSH = _ceil(NLT, CORES)
    LLOC = _ceil(LTSH, P) * P
    NWL = LLOC // P
    lt_owner_all = np.arange(NLT) // LTSH
    lt_local_all = np.arange(NLT) - lt_owner_all * LTSH

    # ---- bill / bill-version sharding -----------------------------------
    nv = np.bincount(bv2bill, minlength=NB)
    cum = np.cumsum(nv)
    starts = cum - nv
    targets = (np.arange(1, CORES) * NBV) // CORES
    bsp = np.searchsorted(cum, targets, side="left") + 1
    bs = np.concatenate([[0], np.minimum(bsp, NB), [NB]])
    bs = np.maximum.accumulate(bs)
    vsort = np.argsort(bv2bill, kind="stable")

    core_blocks = []       # per core: list of (b0, b1)
    for c in range(CORES):
        blocks = []
        b = int(bs[c])
        while b < bs[c + 1]:
            e = b
            vcnt = 0
            while (e < bs[c + 1] and (e - b) < BCAP
                   and vcnt + nv[e] <= VCAP):
                vcnt += nv[e]
                e += 1
            if e == b:
                raise RuntimeError("bill with too many versions for VCAP")
            blocks.append((b, e))
            b = e
        core_blocks.append(blocks)
    nbb = max(len(bl) for bl in core_blocks)
    VLOC = nbb * VCAP
    BLOC = nbb * BCAP
    NWV = VLOC // P

    v_owner = np.zeros(NBV, np.int64)
    v_local = np.zeros(NBV, np.int64)
    bill_local_all = np.zeros(NB, np.int64)
    billloc = np.full((CORES, P, nbb * NVB), -1.0, np.float32)
    tixT = np.zeros((CORES, P, nbb), np.int32)
    rnvT = np.ones((CORES, P, nbb), np.float32)
    bvbillT = np.zeros((CORES, P, NWV), np.int32)
    for c in range(CORES):
        for bb, (b0, b1) in enumerate(core_blocks[c]):
            nb_blk = b1 - b0
            bills = np.arange(b0, b1)
            bill_local_all[bills] = bb * BCAP + np.arange(nb_blk)
            tixT[c, :nb_blk, bb] = topic_ix[bills]
            rnvT[c, :nb_blk, bb] = 1.0 / np.maximum(nv[bills], 1)
            vs = vsort[starts[b0]:starts[b0] + int(nv[bills].sum())]
            nvb = len(vs)
            vl = bb * VCAP + np.arange(nvb)
            v_owner[vs] = c
            v_local[vs] = vl
            billloc[c, vl % P, (vl // P)] = (
                bill_local_all[bv2bill[vs]] - bb * BCAP).astype(np.float32)
            bvbillT[c, vl % P, vl // P] = bill_local_all[bv2bill[vs]]

    # ---- segment-sum relations ------------------------------------------
    rel_don = _prep_segsum(don_src, lt_owner_all[don_dst],
                           lt_local_all[don_dst], NWL)
    rel_lob = _prep_segsum(lob_src, lt_owner_all[lob_dst],
                           lt_local_all[lob_dst], NWL)
    rel_mem = _prep_segsum(m_dst, lt_owner_all[m_src],
                           lt_local_all[m_src], NWL)
    rel_pv = _prep_segsum(pv_src, v_owner[pv_dst],
                          v_local[pv_dst], NWV)
    rel_rd = _prep_segsum(r_dst, v_owner[r_src],
                          v_local[r_src], NWV)

    # reciprocal-count slabs (per core)
    def lt_recips(dst):
        cnts = np.bincount(dst, minlength=NLT)
        out = np.zeros((CORES, P, NWL), np.float32)
        for c in range(CORES):
            lo = c * LTSH
            out[c] = _recipT(cnts[lo:min(lo + LTSH, NLT)], NWL)
        return out

    r_don = lt_recips(don_dst)
    r_lob = lt_recips(lob_dst)
    r_mem = lt_recips(m_src)

    def v_recips(dst):
        cnts = np.bincount(dst, minlength=NBV)
        out = np.zeros((CORES, P, NWV), np.float32)
        for c in range(CORES):
            loc = np.zeros(VLOC, np.int64)
            m = v_owner == c
            loc_cnt = np.zeros(VLOC, np.int64)
            np.add.at(loc_cnt, v_local[m], cnts[m])
            out[c] = _recipT(loc_cnt, NWV)
        return out

    r_pv = v_recips(pv_dst)
    r_rd = v_recips(r_src)

    # ---- vote edges ------------------------------------------------------
    ev_owner = v_owner[vote_bv]
    core_orig = []
    EC = []
    for c in range(CORES):
        ids = np.where(ev_owner == c)[0]
        order = np.argsort(v_local[vote_bv[ids]], kind="stable")
        core_orig.append(ids[order])
        EC.append(len(ids))
    ESLOT = max(_ceil(max(EC), P), 1)
    ESLOT = _ceil(ESLOT, SUP) * SUP
    EROWS = ESLOT * P
    vltT = np.zeros((CORES, P, ESLOT), np.int32)
    vbvT = np.zeros((CORES, P, ESLOT), np.int32)
    lt_gidx_all = lt_owner_all * LLOC + lt_local_all
    for c in range(CORES):
        ids = core_orig[c]
        n = len(ids)
        sl = np.arange(n) // P
        pp = np.arange(n) % P
        vltT[c, pp, sl] = lt_gidx_all[vote_lt[ids]]
        vbvT[c, pp, sl] = v_local[vote_bv[ids]]

    # ---- per-core input maps --------------------------------------------
    hltT = np.zeros((CORES, P, LLOC), np.float32)
    for c in range(CORES):
        lo = c * LTSH
        hi = min(lo + LTSH, NLT)
        hltT[c, :, :hi - lo] = h_lt[lo:hi].T
    htopicT = h_topic.T.copy()                       # [D, NT]
    biasm = np.tile(b_fuse[None, :], (P, 1)).astype(np.float32)
    iota = np.tile(np.arange(P, dtype=np.float32)[None, :],
                   (P, SUP)).reshape(P, SUP * P)
    iota = np.tile(np.arange(P, dtype=np.float32), (P, SUP // 1))
    iota = np.tile(np.arange(P, dtype=np.float32), SUP)[None, :].repeat(P, 0)
    iota = np.ascontiguousarray(iota, np.float32)    # [P, SUP*P]

    in_maps = []
    for c in range(CORES):
        in_maps.append({
            "h_bv": h_bv, "h_donor": h_donor, "h_lobby": h_lobby,
            "h_comm": h_comm,
            "hltT": hltT[c], "htopicT": htopicT,
            "wf": W_fuse, "biasm": biasm, "iota": iota,
            "don_idx": rel_don["idxT"][c], "don_loc": rel_don["locT"][c],
            "lob_idx": rel_lob["idxT"][c], "lob_loc": rel_lob["locT"][c],
            "mem_idx": rel_mem["idxT"][c], "mem_loc": rel_mem["locT"][c],
            "pv_idx": rel_pv["idxT"][c], "pv_loc": rel_pv["locT"][c],
            "rd_idx": rel_rd["idxT"][c], "rd_loc": rel_rd["locT"][c],
            "r_don": r_don[c], "r_lob": r_lob[c], "r_mem": r_mem[c],
            "r_pv": r_pv[c], "r_rd": r_rd[c],
            "billloc": billloc[c], "tixT": tixT[c], "rnv": rnvT[c],
            "bvbill": bvbillT[c],
            "vlt": vltT[c], "vbv": vbvT[c],
        })

    plan = dict(
        NBV=NBV, NLT=NLT, NB=NB, NT=NT, E=E,
        LLOC=LLOC, NWL=NWL, VLOC=VLOC, NWV=NWV, nbb=nbb,
        BLOC=BLOC, ESLOT=ESLOT, EROWS=EROWS,
        ND=h_donor.shape[0], NLF=h_lobby.shape[0], NCM=h_comm.shape[0],
        rels=dict(don=rel_don, lob=rel_lob, mem=rel_mem,
                  pv=rel_pv, rd=rel_rd),
        core_orig=core_orig, EC=EC,
    )
    return plan, in_maps


# ---------------------------------------------------------------------------
# device program
# ---------------------------------------------------------------------------

def _emit_segsum(nc, tc, gpool, opool, pspool, rel, acc, layout,
                 table_ap, idx_sb, loc_sb, iota_sb, rel_name,
                 rscale_sb=None):
    """layout 'dmaj': psum[d, wloc] (lhsT=G, rhs=O);
    'vmaj': psum[wloc, d] (lhsT=O, rhs=G), flushed with per-partition
    recip scale when rscale_sb is given."""
    S = rel["S"]
    wos = rel["wos"]
    nsup = S // SUP
    first = {}
    last = {}
    for s, w in enumerate(wos):
        w = int(w)
        if w not in first:
            first[w] = s
        last[w] = s
    psums = {}
    for su in range(nsup):
        g = gpool.tile([P, SUP * D], F32, tag="g", name=f"g_{rel_name}{su}")
        for j in range(SUP):
            s = su * SUP + j
            nc.gpsimd.indirect_dma_start(
                out=g[:, j * D:(j + 1) * D], out_offset=None, in_=table_ap,
                in_offset=bass.IndirectOffsetOnAxis(
                    ap=idx_sb[:, s:s + 1], axis=0))
        o = opool.tile([P, SUP * P], F32, tag="o", name=f"o_{rel_name}{su}")
        nc.vector.tensor_tensor(
            out=o[:].rearrange("p (s q) -> p s q", q=P),
            in0=iota_sb[:].rearrange("p (s q) -> p s q", q=P),
            in1=_expand_last(loc_sb[:, su * SUP:(su + 1) * SUP], P),
            op=mybir.AluOpType.is_equal)
        for j in range(SUP):
            s = su * SUP + j
            w = int(wos[s])
            if w not in psums:
                psums[w] = pspool.tile([P, P], F32, tag="ps",
                                       name=f"ps_{rel_name}{w}")
            gj = g[:, j * D:(j + 1) * D]
            oj = o[:, j * P:(j + 1) * P]
            if layout == "dmaj":
                nc.tensor.matmul(out=psums[w][:], lhsT=gj, rhs=oj,
                                 start=(s == first[w]), stop=(s == last[w]))
            else:
                nc.tensor.matmul(out=psums[w][:], lhsT=oj, rhs=gj,
                                 start=(s == first[w]), stop=(s == last[w]))
            if s == last[w]:
                dst = acc[:, w * P:(w + 1) * P]
                if rscale_sb is not None:
                    nc.scalar.activation(
                        out=dst, in_=psums[w][:],
                        func=mybir.ActivationFunctionType.Copy,
                        scale=rscale_sb[:, w:w + 1])
                else:
                    nc.vector.tensor_copy(out=dst, in_=psums[w][:])
                del psums[w]


def _build(plan):
    LLOC, NWL = plan["LLOC"], plan["NWL"]
    VLOC, NWV = plan["VLOC"], plan["NWV"]
    nbb, BLOC = plan["nbb"], plan["BLOC"]
    ESLOT, EROWS = plan["ESLOT"], plan["EROWS"]
    NT = plan["NT"]
    rels = plan["rels"]

    nc = bacc.Bacc("TRN2", target_bir_lowering=False, debug=False,
                   num_devices=CORES)

    def din(name, shape, dt=F32):
        return nc.dram_tensor(name, list(shape), dt, kind="ExternalInput")

    t_hbv = din("h_bv", (plan["NBV"], D))
    t_hdon = din("h_donor", (plan["ND"], D))
    t_hlob = din("h_lobby", (plan["NLF"], D))
    t_hcom = din("h_comm", (plan["NCM"], D))
    t_hltT = din("hltT", (P, LLOC))
    t_htopT = din("htopicT", (P, NT))
    t_wf = din("wf", (6 * D, D))
    t_bias = din("biasm", (P, P))
    t_iota = din("iota", (P, SUP * P))
    t_rel = {}
    for rn, rel in rels.items():
        t_rel[rn] = (din(f"{rn}_idx", (P, rel["S"]), I32),
                     din(f"{rn}_loc", (P, rel["S"])))
    t_rdon = din("r_don", (P, NWL))
    t_rlob = din("r_lob", (P, NWL))
    t_rmem = din("r_mem", (P, NWL))
    t_rpv = din("r_pv", (P, NWV))
    t_rrd = din("r_rd", (P, NWV))
    t_billloc = din("billloc", (P, nbb * NVB))
    t_tix = din("tixT", (P, nbb), I32)
    t_rnv = din("rnv", (P, nbb))
    t_bvbill = din("bvbill", (P, NWV), I32)
    t_vlt = din("vlt", (P, ESLOT), I32)
    t_vbv = din("vbv", (P, ESLOT), I32)
    t_out = nc.dram_tensor("out", [EROWS, D], F32, kind="ExternalOutput")

    debug = os.environ.get("BASSK_DEBUG", "0") == "1"
    t_dbg = {}
    if debug:
        t_dbg["ltfull"] = nc.dram_tensor("dbg_ltfull", [CORES * LLOC, D], F32,
                                         kind="ExternalOutput")
        t_dbg["bv"] = nc.dram_tensor("dbg_bv", [VLOC, D], F32,
                                     kind="ExternalOutput")
        t_dbg["bill"] = nc.dram_tensor("dbg_bill", [BLOC, D], F32,
                                       kind="ExternalOutput")
        t_dbg["topw5"] = nc.dram_tensor("dbg_topw5", [NT, D], F32,
                                        kind="ExternalOutput")
        t_dbg["accrd"] = nc.dram_tensor("dbg_accrd", [P, NWV * P], F32,
                                        kind="ExternalOutput")
        t_dbg["accpv"] = nc.dram_tensor("dbg_accpv", [P, VLOC], F32,
                                        kind="ExternalOutput")
        t_dbg["accdon"] = nc.dram_tensor("dbg_accdon", [P, LLOC], F32,
                                         kind="ExternalOutput")

    Copy = mybir.ActivationFunctionType.Copy

    with tile.TileContext(nc) as tc:
        with (
            tc.tile_pool(name="persist", bufs=1) as pp,
            tc.tile_pool(name="gpool", bufs=4) as gpool,
            tc.tile_pool(name="opool", bufs=3) as opool,
            tc.tile_pool(name="spool", bufs=4) as spool,
            tc.tile_pool(name="pspool", bufs=8, space="PSUM") as pspool,
            tc.tile_pool(name="dram", bufs=1, space="DRAM") as dram,
        ):
            def load(t, shape, dt=F32, name=None):
                sb = pp.tile(list(shape), dt, name=name or (t.name + "_sb"))
                nc.sync.dma_start(out=sb[:], in_=t.ap())
                return sb

            iota_sb = load(t_iota, (P, SUP * P))
            bias_sb = load(t_bias, (P, P))
            hltT_sb = load(t_hltT, (P, LLOC))
            htopT_sb = load(t_htopT, (P, NT))
            w_sb = []
            for k in range(6):
                wsb = pp.tile([P, D], F32, name=f"w{k}_sb")
                nc.sync.dma_start(out=wsb[:], in_=t_wf.ap()[k * D:(k + 1) * D, :])
                w_sb.append(wsb)
            rdon_sb = load(t_rdon, (P, NWL))
            rlob_sb = load(t_rlob, (P, NWL))
            rmem_sb = load(t_rmem, (P, NWL))
            rpv_sb = load(t_rpv, (P, NWV))
            rrd_sb = load(t_rrd, (P, NWV))
            billloc_sb = load(t_billloc, (P, nbb * NVB))
            tix_sb = load(t_tix, (P, nbb), I32)
            rnv_sb = load(t_rnv, (P, nbb))
            bvbill_sb = load(t_bvbill, (P, NWV), I32)
            vlt_sb = load(t_vlt, (P, ESLOT), I32)
            vbv_sb = load(t_vbv, (P, ESLOT), I32)
            rel_sb = {}
            for rn, rel in rels.items():
                rel_sb[rn] = (load(t_rel[rn][0], (P, rel["S"]), I32),
                              load(t_rel[rn][1], (P, rel["S"])))

            # DRAM intermediates
            topw5_dram = dram.tile([NT, D], F32, name="topw5_dram")
            bill_dram = dram.tile([BLOC, D], F32, name="bill_dram")
            bv_dram = dram.tile([VLOC, D], F32, name="bv_dram")
            ltb_dram = dram.tile([LLOC, D], F32, name="ltb_dram")
            ltfull_dram = dram.tile([CORES * LLOC, D], F32,
                                    addr_space="Shared", name="ltfull_dram")

            # topicW5 = h_topic @ W5 -> DRAM
            ptw = pspool.tile([P, P], F32, tag="ps", name="ptw")
            nc.tensor.matmul(out=ptw[:NT, :], lhsT=htopT_sb[:, :NT],
                             rhs=w_sb[5][:], start=True, stop=True)
            stw = spool.tile([P, D], F32, tag="t", name="stw")
            nc.vector.tensor_copy(out=stw[:NT, :], in_=ptw[:NT, :])
            nc.sync.dma_start(out=topw5_dram[:], in_=stw[:NT, :])

            # ---- LT-space segment means ---------------------------------
            acc_don = pp.tile([P, LLOC], F32, name="acc_don")
            acc_lob = pp.tile([P, LLOC], F32, name="acc_lob")
            acc_mem = pp.tile([P, LLOC], F32, name="acc_mem")
            nc.vector.memset(acc_don[:], 0.0)
            nc.vector.memset(acc_lob[:], 0.0)
            nc.vector.memset(acc_mem[:], 0.0)
            _emit_segsum(nc, tc, gpool, opool, pspool, rels["don"], acc_don,
                         "dmaj", t_hdon.ap(), *rel_sb["don"], iota_sb, "don")
            _emit_segsum(nc, tc, gpool, opool, pspool, rels["lob"], acc_lob,
                         "dmaj", t_hlob.ap(), *rel_sb["lob"], iota_sb, "lob")
            _emit_segsum(nc, tc, gpool, opool, pspool, rels["mem"], acc_mem,
                         "dmaj", t_hcom.ap(), *rel_sb["mem"], iota_sb, "mem")

            # LT table blocks -> ltb_dram, then AllGather
            for lb in range(NWL):
                sl = slice(lb * P, (lb + 1) * P)
                p0 = pspool.tile([P, P], F32, tag="ps", name=f"plt0_{lb}")
                nc.tensor.matmul(out=p0[:], lhsT=hltT_sb[:, sl],
                                 rhs=w_sb[0][:], start=True, stop=True)
                pm = pspool.tile([P, P], F32, tag="ps", name=f"pltm_{lb}")
                nc.tensor.matmul(out=pm[:], lhsT=acc_mem[:, sl],
                                 rhs=w_sb[3][:], start=True, stop=True)
                sm = spool.tile([P, P], F32, tag="t", name=f"sltm_{lb}")
                nc.scalar.activation(out=sm[:], in_=pm[:], func=Copy,
                                     scale=rmem_sb[:, lb:lb + 1])
                pd = pspool.tile([P, P], F32, tag="ps", name=f"pltd_{lb}")
                nc.tensor.matmul(out=pd[:], lhsT=acc_don[:, sl],
                                 rhs=w_sb[4][:], start=True, stop=True)
                sd = spool.tile([P, P], F32, tag="t", name=f"sltd_{lb}")
                nc.scalar.activation(out=sd[:], in_=pd[:], func=Copy,
                                     scale=rdon_sb[:, lb:lb + 1])
                pl = pspool.tile([P, P], F32, tag="ps", name=f"pltl_{lb}")
                nc.tensor.matmul(out=pl[:], lhsT=acc_lob[:, sl],
                                 rhs=w_sb[4][:], start=True, stop=True)
                sl2 = spool.tile([P, P], F32, tag="t", name=f"sltl_{lb}")
                nc.scalar.activation(out=sl2[:], in_=pl[:], func=Copy,
                                     scale=rlob_sb[:, lb:lb + 1])
                tt = spool.tile([P, P], F32, tag="t2", name=f"tlt_{lb}")
                nc.vector.tensor_add(out=tt[:], in0=p0[:], in1=sm[:])
                nc.vector.tensor_add(out=tt[:], in0=tt[:], in1=sd[:])
                nc.vector.tensor_add(out=tt[:], in0=tt[:], in1=sl2[:])
                nc.vector.tensor_add(out=tt[:], in0=tt[:], in1=bias_sb[:])
                nc.sync.dma_start(out=ltb_dram[lb * P:(lb + 1) * P, :],
                                  in_=tt[:])
            nc.gpsimd.collective_compute(
                "AllGather", mybir.AluOpType.bypass,
                replica_groups=[list(range(CORES))],
                ins=[ltb_dram.opt()], outs=[ltfull_dram.opt()])

            # ---- rd segment means (v-major) + bill table ----------------
            acc_rd = pp.tile([P, NWV * P], F32, name="acc_rd")
            nc.vector.memset(acc_rd[:], 0.0)
            _emit_segsum(nc, tc, gpool, opool, pspool, rels["rd"], acc_rd,
                         "vmaj", t_hcom.ap(), *rel_sb["rd"], iota_sb, "rd",
                         rscale_sb=rrd_sb)
            for bb in range(nbb):
                pbc = pspool.tile([P, P], F32, tag="ps", name=f"pbc_{bb}")
                for i in range(NVB):
                    col = bb * NVB + i
                    ob = opool.tile([P, P], F32, tag="ob", name=f"ob_{bb}_{i}")
                    nc.vector.tensor_tensor(
                        out=ob[:], in0=iota_sb[:, :P],
                        in1=billloc_sb[:, col:col + 1].to_broadcast([P, P]),
                        op=mybir.AluOpType.is_equal)
                    vb = bb * NVB + i
                    nc.tensor.matmul(
                        out=pbc[:], lhsT=acc_rd[:, vb * P:(vb + 1) * P],
                        rhs=ob[:], start=(i == 0), stop=(i == NVB - 1))
                bc = spool.tile([P, P], F32, tag="t", name=f"bc_{bb}")
                nc.vector.tensor_copy(out=bc[:], in_=pbc[:])
                pbt = pspool.tile([P, P], F32, tag="ps", name=f"pbt_{bb}")
                nc.tensor.matmul(out=pbt[:], lhsT=bc[:], rhs=w_sb[2][:],
                                 start=True, stop=True)
                bt = spool.tile([P, P], F32, tag="t2", name=f"bt_{bb}")
                nc.scalar.activation(out=bt[:], in_=pbt[:], func=Copy,
                                     scale=rnv_sb[:, bb:bb + 1])
                tg = gpool.tile([P, D], F32, tag="gs", name=f"tg_{bb}")
                nc.gpsimd.indirect_dma_start(
                    out=tg[:], out_offset=None, in_=topw5_dram[:],
                    in_offset=bass.IndirectOffsetOnAxis(
                        ap=tix_sb[:, bb:bb + 1], axis=0))
                nc.vector.tensor_add(out=bt[:], in0=bt[:], in1=tg[:])
                nc.sync.dma_start(out=bill_dram[bb * P:(bb + 1) * P, :],
                                  in_=bt[:])

            # ---- pv segment means + BV table ----------------------------
            acc_pv = pp.tile([P, VLOC], F32, name="acc_pv")
            nc.vector.memset(acc_pv[:], 0.0)
            _emit_segsum(nc, tc, gpool, opool, pspool, rels["pv"], acc_pv,
                         "dmaj", t_hbv.ap(), *rel_sb["pv"], iota_sb, "pv")
            for vb in range(NWV):
                ppv = pspool.tile([P, P], F32, tag="ps", name=f"ppv_{vb}")
                nc.tensor.matmul(out=ppv[:],
                                 lhsT=acc_pv[:, vb * P:(vb + 1) * P],
                                 rhs=w_sb[1][:], start=True, stop=True)
                sv = spool.tile([P, P], F32, tag="t", name=f"sv_{vb}")
                nc.scalar.activation(out=sv[:], in_=ppv[:], func=Copy,
                                     scale=rpv_sb[:, vb:vb + 1])
                gb = gpool.tile([P, D], F32, tag="gs", name=f"gb_{vb}")
                nc.gpsimd.indirect_dma_start(
                    out=gb[:], out_offset=None, in_=bill_dram[:],
                    in_offset=bass.IndirectOffsetOnAxis(
                        ap=bvbill_sb[:, vb:vb + 1], axis=0))
                nc.vector.tensor_add(out=sv[:], in0=sv[:], in1=gb[:])
                nc.sync.dma_start(out=bv_dram[vb * P:(vb + 1) * P, :],
                                  in_=sv[:])

            # ---- final edge pass ----------------------------------------
            for su in range(ESLOT // SUP):
                glt = gpool.tile([P, SUP * D], F32, tag="g",
                                 name=f"glt_{su}")
                gbv = gpool.tile([P, SUP * D], F32, tag="g",
                                 name=f"gbv_{su}")
                for j in range(SUP):
                    s = su * SUP + j
                    nc.gpsimd.indirect_dma_start(
                        out=glt[:, j * D:(j + 1) * D], out_offset=None,
                        in_=ltfull_dram[:],
                        in_offset=bass.IndirectOffsetOnAxis(
                            ap=vlt_sb[:, s:s + 1], axis=0))
                    nc.gpsimd.indirect_dma_start(
                        out=gbv[:, j * D:(j + 1) * D], out_offset=None,
                        in_=bv_dram[:],
                        in_offset=bass.IndirectOffsetOnAxis(
                            ap=vbv_sb[:, s:s + 1], axis=0))
                nc.vector.tensor_add(out=glt[:], in0=glt[:], in1=gbv[:])
                nc.sync.dma_start(
                    out=t_out.ap()[su * SUP * P:(su + 1) * SUP * P, :]
                    .rearrange("(g p) d -> p g d", p=P),
                    in_=glt[:].rearrange("p (g d) -> p g d", d=D))

            if debug:
                nc.sync.dma_start(out=t_dbg["ltfull"].ap(),
                                  in_=ltfull_dram[:])
                nc.sync.dma_start(out=t_dbg["bv"].ap(), in_=bv_dram[:])
                nc.sync.dma_start(out=t_dbg["bill"].ap(), in_=bill_dram[:])
                nc.sync.dma_start(out=t_dbg["topw5"].ap(), in_=topw5_dram[:])
                nc.sync.dma_start(out=t_dbg["accrd"].ap(), in_=acc_rd[:])
                nc.sync.dma_start(out=t_dbg["accpv"].ap(), in_=acc_pv[:])
                nc.sync.dma_start(out=t_dbg["accdon"].ap(), in_=acc_don[:])

    nc.compile()
    return nc


# ---------------------------------------------------------------------------
# entry point
# ---------------------------------------------------------------------------

def kernel(**inputs):
    global _LAST_EXEC_NS
    plan, in_maps = _prep(inputs)
    nc = _build(plan)

    from concourse import bass_utils
    trace = os.environ.get("BASSK_TRACE", "0") == "1"
    if trace:
        try:
            import ntff_shim  # noqa: F401
        except ImportError:
            pass
    res = bass_utils.run_bass_kernel_spmd(
        nc, in_maps, core_ids=list(range(CORES)), trace=trace)
    _LAST_EXEC_NS = res.exec_time_ns

    E = plan["E"]
    out = np.zeros((E, D), np.float32)
    for c in range(CORES):
        ids = plan["core_orig"][c]
        out[ids] = res.results[c]["out"][:len(ids)]
    return out


# revision 12
# speedup vs baseline: 1.0732x; 1.0732x over previous
"""MetaPathAgg Trainium2 kernel (8 NeuronCores, SPMD).

Algebraic restructuring: out[e] = LT_table[vote_lt[e]] + BV_table[vote_bv[e]]
where
  LT_table = h_lt @ W0 + mean_mem @ W3 + (mean_don + mean_lob) @ W4 + b_fuse
  BV_table[v] = mean_pv[v] @ W1 + bill_table[bv2bill[v]]
  bill_table[b] = bill_comm[b] @ W2 + (h_topic @ W5)[topic_ix[b]]
  bill_comm[b] = mean over versions v of b of mean_rd[v]

All segment-means run as: indirect-DMA row gather + is_equal one-hot +
PSUM-accumulated matmuls, destination-sharded over 8 cores so no
all-reduce is needed (only a 2.5 MB LT-table AllGather).
"""

import math
import os
import sys

import numpy as np

sys.path.insert(0, "/opt/trn_rl_repo")

import concourse.bass as bass  # noqa: E402
import concourse.bacc as bacc  # noqa: E402
import concourse.mybir as mybir  # noqa: E402
import concourse.tile as tile  # noqa: E402

CORES = 8
P = 128
D = 128
SUP = 16       # gather slots batched per indirect DMA / one-hot op
VCAP = 256     # bill-versions per bill-block (2 windows of 128)
BCAP = 128     # bills per bill-block
NVB = VCAP // P

F32 = mybir.dt.float32
I32 = mybir.dt.int32

_LAST_EXEC_NS = None


def _expand_last(ap, n):
    """[.., k] AP -> [.., k, n] with a step-0 broadcast dim appended."""
    return bass.AP(ap.tensor, ap.offset, list(ap.ap) + [[0, n]])


def _ceil(a, b):
    return (a + b - 1) // b


# ---------------------------------------------------------------------------
# host-side integer preprocessing
# ---------------------------------------------------------------------------

def _prep_segsum(feat_idx, owner, local, nwin):
    """Pack edges of one relation into per-core window-aligned 128-slots.

    Returns idxT [CORES,P,SPAD] int32 (gather row ids; pads point at row 0),
    locT [CORES,P,SPAD] f32 (dst offset within its 128-window; pads -1),
    win_of_slot [SPAD].
    """
    cnt = np.zeros((CORES, nwin), np.int64)
    np.add.at(cnt, (owner, local // P), 1)
    nslot_w = _ceil(cnt, P).max(axis=0)
    slot_base = np.concatenate([[0], np.cumsum(nslot_w)]).astype(np.int64)
    S = int(slot_base[-1])
    SPAD = max(_ceil(S, SUP) * SUP, SUP)
    win_of_slot = np.repeat(np.arange(nwin), nslot_w)
    win_of_slot = np.concatenate(
        [win_of_slot, np.full(SPAD - S, max(nwin - 1, 0))]).astype(np.int64)
    idxT = np.zeros((CORES, P, SPAD), np.int32)
    locT = np.full((CORES, P, SPAD), -1.0, np.float32)
    for c in range(CORES):
        m = owner == c
        fi = feat_idx[m]
        lo = local[m]
        order = np.argsort(lo, kind="stable")
        fi = fi[order]
        lo = lo[order]
        w = lo // P
        wstart = np.searchsorted(w, np.arange(nwin))
        r = np.arange(len(lo)) - wstart[w]
        pos = slot_base[w] * P + r
        slot = pos // P
        part = pos % P
        idxT[c, part, slot] = fi
        locT[c, part, slot] = (lo - w * P).astype(np.float32)
    return dict(idxT=idxT, locT=locT, wos=win_of_slot, S=SPAD)


def _recipT(counts_local, nwin):
    """counts over local dst space -> [P, nwin] per-partition recip slab."""
    r = np.ones(nwin * P, np.float32)
    n = len(counts_local)
    r[:n] = 1.0 / np.maximum(counts_local, 1)
    return r.reshape(nwin, P).T.copy()


def _prep(inputs):
    h_bv = np.asarray(inputs["h_bv"], np.float32)
    h_lt = np.asarray(inputs["h_lt"], np.float32)
    h_comm = np.asarray(inputs["h_comm"], np.float32)
    h_donor = np.asarray(inputs["h_donor"], np.float32)
    h_lobby = np.asarray(inputs["h_lobby"], np.float32)
    h_topic = np.asarray(inputs["h_topic"], np.float32)
    W_fuse = np.asarray(inputs["W_fuse"], np.float32)
    b_fuse = np.asarray(inputs["b_fuse"], np.float32)
    vote_lt = np.asarray(inputs["vote_lt"]).astype(np.int64)
    vote_bv = np.asarray(inputs["vote_bv"]).astype(np.int64)
    bv2bill = np.asarray(inputs["bv2bill"]).astype(np.int64)
    topic_ix = np.asarray(inputs["topic_ix"]).astype(np.int64)
    pv_src = np.asarray(inputs["pv_src"]).astype(np.int64)
    pv_dst = np.asarray(inputs["pv_dst"]).astype(np.int64)
    r_src = np.asarray(inputs["r_src"]).astype(np.int64)
    r_dst = np.asarray(inputs["r_dst"]).astype(np.int64)
    m_src = np.asarray(inputs["m_src"]).astype(np.int64)
    m_dst = np.asarray(inputs["m_dst"]).astype(np.int64)
    don_src = np.asarray(inputs["don_src"]).astype(np.int64)
    don_dst = np.asarray(inputs["don_dst"]).astype(np.int64)
    lob_src = np.asarray(inputs["lob_src"]).astype(np.int64)
    lob_dst = np.asarray(inputs["lob_dst"]).astype(np.int64)

    NBV = h_bv.shape[0]
    NLT = h_lt.shape[0]
    NB = np.asarray(inputs["h_bill"]).shape[0]
    NT = h_topic.shape[0]
    E = vote_lt.shape[0]
    assert NT <= P

    # ---- LT sharding -----------------------------------------------------
    LTSH = _ceil(NLT, CORES)
    LLOC = _ceil(LTSH, P) * P
    NWL = LLOC // P
    lt_owner_all = np.arange(NLT) // LTSH
    lt_local_all = np.arange(NLT) - lt_owner_all * LTSH

    # ---- bill / bill-version sharding -----------------------------------
    nv = np.bincount(bv2bill, minlength=NB)
    cum = np.cumsum(nv)
    starts = cum - nv
    targets = (np.arange(1, CORES) * NBV) // CORES
    bsp = np.searchsorted(cum, targets, side="left") + 1
    bs = np.concatenate([[0], np.minimum(bsp, NB), [NB]])
    bs = np.maximum.accumulate(bs)
    vsort = np.argsort(bv2bill, kind="stable")

    core_blocks = []       # per core: list of (b0, b1)
    for c in range(CORES):
        blocks = []
        b = int(bs[c])
        while b < bs[c + 1]:
            e = b
            vcnt = 0
            while (e < bs[c + 1] and (e - b) < BCAP
                   and vcnt + nv[e] <= VCAP):
                vcnt += nv[e]
                e += 1
            if e == b:
                raise RuntimeError("bill with too many versions for VCAP")
            blocks.append((b, e))
            b = e
        core_blocks.append(blocks)
    nbb = max(len(bl) for bl in core_blocks)
    VLOC = nbb * VCAP
    BLOC = nbb * BCAP
    NWV = VLOC // P

    v_owner = np.zeros(NBV, np.int64)
    v_local = np.zeros(NBV, np.int64)
    bill_local_all = np.zeros(NB, np.int64)
    billloc = np.full((CORES, P, nbb * NVB), -1.0, np.float32)
    tixF = np.full((CORES, P, nbb), -1.0, np.float32)
    rnvT = np.ones((CORES, P, nbb), np.float32)
    for c in range(CORES):
        for bb, (b0, b1) in enumerate(core_blocks[c]):
            nb_blk = b1 - b0
            bills = np.arange(b0, b1)
            bill_local_all[bills] = bb * BCAP + np.arange(nb_blk)
            tixF[c, :nb_blk, bb] = topic_ix[bills].astype(np.float32)
            rnvT[c, :nb_blk, bb] = 1.0 / np.maximum(nv[bills], 1)
            vs = vsort[starts[b0]:starts[b0] + int(nv[bills].sum())]
            nvb = len(vs)
            vl = bb * VCAP + np.arange(nvb)
            v_owner[vs] = c
            v_local[vs] = vl
            billloc[c, vl % P, (vl // P)] = (
                bill_local_all[bv2bill[vs]] - bb * BCAP).astype(np.float32)

    # ---- segment-sum relations ------------------------------------------
    rel_don = _prep_segsum(don_src, lt_owner_all[don_dst],
                           lt_local_all[don_dst], NWL)
    rel_lob = _prep_segsum(lob_src, lt_owner_all[lob_dst],
                           lt_local_all[lob_dst], NWL)
    rel_pv = _prep_segsum(pv_src, v_owner[pv_dst],
                          v_local[pv_dst], NWV)
    NCM = h_comm.shape[0]
    CH = _ceil(NCM, P)  # committee halves (2 for NCM=200)
    VLOC_ = VLOC
    rdC = np.zeros((CORES, CH * P, VLOC_), np.float32)
    np.add.at(rdC, (v_owner[r_src], r_dst, v_local[r_src]), 1.0)
    cnt_rd = np.zeros((CORES, VLOC_), np.float32)
    np.add.at(cnt_rd, (v_owner[r_src], v_local[r_src]), 1.0)
    rdC /= np.maximum(cnt_rd, 1.0)[:, None, :]
    rdC = rdC.reshape(CORES, CH, P, NWV, P).transpose(0, 2, 3, 1, 4)
    rdC = np.ascontiguousarray(rdC.reshape(CORES, P, NWV * CH * P))
    memC = np.zeros((CORES, CH * P, LLOC), np.float32)
    np.add.at(memC, (lt_owner_all[m_src], m_dst, lt_local_all[m_src]), 1.0)
    cnt_mem = np.zeros((CORES, LLOC), np.float32)
    np.add.at(cnt_mem, (lt_owner_all[m_src], lt_local_all[m_src]), 1.0)
    memC /= np.maximum(cnt_mem, 1.0)[:, None, :]
    memC = memC.reshape(CORES, CH, P, NWL, P).transpose(0, 2, 3, 1, 4)
    memC = np.ascontiguousarray(memC.reshape(CORES, P, NWL * CH * P))

    # reciprocal-count slabs (per core)
    def lt_recips(dst):
        cnts = np.bincount(dst, minlength=NLT)
        out = np.zeros((CORES, P, NWL), np.float32)
        for c in range(CORES):
            lo = c * LTSH
            out[c] = _recipT(cnts[lo:min(lo + LTSH, NLT)], NWL)
        return out

    r_don = lt_recips(don_dst)
    r_lob = lt_recips(lob_dst)

    def v_recips(dst):
        cnts = np.bincount(dst, minlength=NBV)
        out = np.zeros((CORES, P, NWV), np.float32)
        for c in range(CORES):
            loc = np.zeros(VLOC, np.int64)
            m = v_owner == c
            loc_cnt = np.zeros(VLOC, np.int64)
            np.add.at(loc_cnt, v_local[m], cnts[m])
            out[c] = _recipT(loc_cnt, NWV)
        return out

    r_pv = v_recips(pv_dst)

    # ---- vote edges ------------------------------------------------------
    ev_owner = v_owner[vote_bv]
    core_orig = []
    EC = []
    for c in range(CORES):
        ids = np.where(ev_owner == c)[0]
        order = np.argsort(v_local[vote_bv[ids]], kind="stable")
        core_orig.append(ids[order])
        EC.append(len(ids))
    ESLOT = max(_ceil(max(EC), P), 1)
    ESLOT = _ceil(ESLOT, SUP) * SUP
    EROWS = ESLOT * P
    vltT = np.zeros((CORES, P, ESLOT), np.int32)
    vbvT = np.zeros((CORES, P, ESLOT), np.int32)
    lt_gidx_all = lt_owner_all * LLOC + lt_local_all
    for c in range(CORES):
        ids = core_orig[c]
        n = len(ids)
        sl = np.arange(n) // P
        pp = np.arange(n) % P
        vltT[c, pp, sl] = lt_gidx_all[vote_lt[ids]]
        vbvT[c, pp, sl] = v_local[vote_bv[ids]]

    # ---- per-core input maps --------------------------------------------
    hltT = np.zeros((CORES, P, LLOC), np.float32)
    for c in range(CORES):
        lo = c * LTSH
        hi = min(lo + LTSH, NLT)
        hltT[c, :, :hi - lo] = h_lt[lo:hi].T
    htopicT = h_topic.T.copy()                       # [D, NT]
    biasm = np.tile(b_fuse[None, :], (P, 1)).astype(np.float32)
    iota = np.tile(np.arange(P, dtype=np.float32)[None, :],
                   (P, SUP)).reshape(P, SUP * P)
    iota = np.tile(np.arange(P, dtype=np.float32), (P, SUP // 1))
    iota = np.tile(np.arange(P, dtype=np.float32), SUP)[None, :].repeat(P, 0)
    iota = np.ascontiguousarray(iota, np.float32)    # [P, SUP*P]

    in_maps = []
    for c in range(CORES):
        in_maps.append({
            "h_bv": h_bv, "h_donor": h_donor, "h_lobby": h_lobby,
            "h_comm": h_comm,
            "hltT": hltT[c], "htopicT": htopicT,
            "wf": W_fuse, "biasm": biasm, "iota": iota,
            "don_idx": rel_don["idxT"][c], "don_loc": rel_don["locT"][c],
            "lob_idx": rel_lob["idxT"][c], "lob_loc": rel_lob["locT"][c],
            "pv_idx": rel_pv["idxT"][c], "pv_loc": rel_pv["locT"][c],
            "rdC": rdC[c], "memC": memC[c],
            "r_don": r_don[c], "r_lob": r_lob[c],
            "r_pv": r_pv[c],
            "billloc": billloc[c], "tixF": tixF[c], "rnv": rnvT[c],
            "vlt": vltT[c], "vbv": vbvT[c],
        })

    plan = dict(
        NBV=NBV, NLT=NLT, NB=NB, NT=NT, E=E,
        LLOC=LLOC, NWL=NWL, VLOC=VLOC, NWV=NWV, nbb=nbb,
        BLOC=BLOC, ESLOT=ESLOT, EROWS=EROWS,
        ND=h_donor.shape[0], NLF=h_lobby.shape[0], NCM=NCM,
        rels=dict(don=rel_don, lob=rel_lob, pv=rel_pv),
        CH=CH,
        core_orig=core_orig, EC=EC,
    )
    return plan, in_maps


# ---------------------------------------------------------------------------
# device program
# ---------------------------------------------------------------------------

def _emit_segsum(nc, tc, gpool, opool, pspool, rel, acc, layout,
                 table_ap, idx_sb, loc_sb, iota_sb, rel_name,
                 rscale_sb=None):
    """layout 'dmaj': psum[d, wloc] (lhsT=G, rhs=O);
    'vmaj': psum[wloc, d] (lhsT=O, rhs=G), flushed with per-partition
    recip scale when rscale_sb is given."""
    S = rel["S"]
    wos = rel["wos"]
    nsup = S // SUP
    first = {}
    last = {}
    for s, w in enumerate(wos):
        w = int(w)
        if w not in first:
            first[w] = s
        last[w] = s
    psums = {}
    for su in range(nsup):
        g = gpool.tile([P, SUP * D], F32, tag="g", name=f"g_{rel_name}{su}")
        for j in range(SUP):
            s = su * SUP + j
            nc.gpsimd.indirect_dma_start(
                out=g[:, j * D:(j + 1) * D], out_offset=None, in_=table_ap,
                in_offset=bass.IndirectOffsetOnAxis(
                    ap=idx_sb[:, s:s + 1], axis=0))
        o = opool.tile([P, SUP * P], F32, tag="o", name=f"o_{rel_name}{su}")
        nc.vector.tensor_tensor(
            out=o[:].rearrange("p (s q) -> p s q", q=P),
            in0=iota_sb[:].rearrange("p (s q) -> p s q", q=P),
            in1=_expand_last(loc_sb[:, su * SUP:(su + 1) * SUP], P),
            op=mybir.AluOpType.is_equal)
        for j in range(SUP):
            s = su * SUP + j
            w = int(wos[s])
            if w not in psums:
                psums[w] = pspool.tile([P, P], F32, tag="ps",
                                       name=f"ps_{rel_name}{w}")
            gj = g[:, j * D:(j + 1) * D]
            oj = o[:, j * P:(j + 1) * P]
            if layout == "dmaj":
                nc.tensor.matmul(out=psums[w][:], lhsT=gj, rhs=oj,
                                 start=(s == first[w]), stop=(s == last[w]))
            else:
                nc.tensor.matmul(out=psums[w][:], lhsT=oj, rhs=gj,
                                 start=(s == first[w]), stop=(s == last[w]))
            if s == last[w]:
                dst = acc[:, w * P:(w + 1) * P]
                if rscale_sb is not None:
                    nc.scalar.activation(
                        out=dst, in_=psums[w][:],
                        func=mybir.ActivationFunctionType.Copy,
                        scale=rscale_sb[:, w:w + 1])
                else:
                    nc.vector.tensor_copy(out=dst, in_=psums[w][:])
                del psums[w]


def _build(plan):
    LLOC, NWL = plan["LLOC"], plan["NWL"]
    VLOC, NWV = plan["VLOC"], plan["NWV"]
    nbb, BLOC = plan["nbb"], plan["BLOC"]
    ESLOT, EROWS = plan["ESLOT"], plan["EROWS"]
    NT = plan["NT"]
    rels = plan["rels"]

    nc = bacc.Bacc("TRN2", target_bir_lowering=False, debug=False,
                   num_devices=CORES)

    def din(name, shape, dt=F32):
        return nc.dram_tensor(name, list(shape), dt, kind="ExternalInput")

    t_hbv = din("h_bv", (plan["NBV"], D))
    t_hdon = din("h_donor", (plan["ND"], D))
    t_hlob = din("h_lobby", (plan["NLF"], D))
    t_hcom = din("h_comm", (plan["NCM"], D))
    t_hltT = din("hltT", (P, LLOC))
    t_htopT = din("htopicT", (P, NT))
    t_wf = din("wf", (6 * D, D))
    t_bias = din("biasm", (P, P))
    t_iota = din("iota", (P, SUP * P))
    t_rel = {}
    for rn, rel in rels.items():
        t_rel[rn] = (din(f"{rn}_idx", (P, rel["S"]), I32),
                     din(f"{rn}_loc", (P, rel["S"])))
    t_rdon = din("r_don", (P, NWL))
    t_rlob = din("r_lob", (P, NWL))
    t_rpv = din("r_pv", (P, NWV))
    CH = plan["CH"]
    NCM = plan["NCM"]
    t_rdC = din("rdC", (P, NWV * CH * P))
    t_memC = din("memC", (P, NWL * CH * P))
    t_billloc = din("billloc", (P, nbb * NVB))
    t_tixf = din("tixF", (P, nbb))
    t_rnv = din("rnv", (P, nbb))
    t_vlt = din("vlt", (P, ESLOT), I32)
    t_vbv = din("vbv", (P, ESLOT), I32)
    t_out = nc.dram_tensor("out", [EROWS, D], F32, kind="ExternalOutput")

    debug = os.environ.get("BASSK_DEBUG", "0") == "1"
    t_dbg = {}
    if debug:
        t_dbg["ltfull"] = nc.dram_tensor("dbg_ltfull", [CORES * LLOC, D], F32,
                                         kind="ExternalOutput")
        t_dbg["bv"] = nc.dram_tensor("dbg_bv", [VLOC, D], F32,
                                     kind="ExternalOutput")
        t_dbg["accrd"] = nc.dram_tensor("dbg_accrd", [P, NWV * P], F32,
                                        kind="ExternalOutput")
        t_dbg["accpv"] = nc.dram_tensor("dbg_accpv", [P, VLOC], F32,
                                        kind="ExternalOutput")
        t_dbg["accdon"] = nc.dram_tensor("dbg_accdon", [P, LLOC], F32,
                                         kind="ExternalOutput")

    Copy = mybir.ActivationFunctionType.Copy

    with tile.TileContext(nc) as tc:
        with (
            tc.tile_pool(name="persist", bufs=1) as pp,
            tc.tile_pool(name="gpool", bufs=3) as gpool,
            tc.tile_pool(name="opool", bufs=2) as opool,
            tc.tile_pool(name="spool", bufs=4) as spool,
            tc.tile_pool(name="pspool", bufs=8, space="PSUM") as pspool,
            tc.tile_pool(name="dram", bufs=1, space="DRAM") as dram,
        ):
            def load(t, shape, dt=F32, name=None):
                sb = pp.tile(list(shape), dt, name=name or (t.name + "_sb"))
                nc.sync.dma_start(out=sb[:], in_=t.ap())
                return sb

            iota_sb = load(t_iota, (P, SUP * P))
            bias_sb = load(t_bias, (P, P))
            hltT_sb = load(t_hltT, (P, LLOC))
            htopT_sb = load(t_htopT, (P, NT))
            w_sb = []
            for k in range(6):
                wsb = pp.tile([P, D], F32, name=f"w{k}_sb")
                nc.sync.dma_start(out=wsb[:], in_=t_wf.ap()[k * D:(k + 1) * D, :])
                w_sb.append(wsb)
            rdon_sb = load(t_rdon, (P, NWL))
            rlob_sb = load(t_rlob, (P, NWL))
            rpv_sb = load(t_rpv, (P, NWV))
            billloc_sb = load(t_billloc, (P, nbb * NVB))
            tixf_sb = load(t_tixf, (P, nbb))
            rnv_sb = load(t_rnv, (P, nbb))
            # committee rows on partitions, two halves (NCM <= 256)
            hc = []
            for h in range(CH):
                t = pp.tile([P, D], F32, name=f"hc{h}_sb")
                lo = h * P
                hi = min(lo + P, NCM)
                if hi - lo < P:
                    nc.vector.memset(t[:], 0.0)
                nc.sync.dma_start(out=t[:hi - lo, :],
                                  in_=t_hcom.ap()[lo:hi, :])
                hc.append(t)
            from concourse.masks import make_identity
            ident_sb = pp.tile([P, P], F32, name="ident_sb")
            make_identity(nc, ident_sb[:])
            bill_sb = pp.tile([P, nbb * D], F32, name="bill_sb")
            topw5_sb = pp.tile([P, D], F32, name="topw5_sb")
            nc.vector.memset(topw5_sb[:], 0.0)
            vlt_sb = load(t_vlt, (P, ESLOT), I32)
            vbv_sb = load(t_vbv, (P, ESLOT), I32)
            rel_sb = {}
            for rn, rel in rels.items():
                rel_sb[rn] = (load(t_rel[rn][0], (P, rel["S"]), I32),
                              load(t_rel[rn][1], (P, rel["S"])))

            # DRAM intermediates
            bv_dram = dram.tile([VLOC, D], F32, name="bv_dram")
            ltb_dram = dram.tile([LLOC, D], F32, name="ltb_dram")
            ltfull_dram = dram.tile([CORES * LLOC, D], F32,
                                    addr_space="Shared", name="ltfull_dram")

            # topicW5 = h_topic @ W5 -> SBUF (rows on partitions)
            ptw = pspool.tile([P, P], F32, tag="ps", name="ptw")
            nc.tensor.matmul(out=ptw[:NT, :], lhsT=htopT_sb[:, :NT],
                             rhs=w_sb[5][:], start=True, stop=True)
            nc.vector.tensor_copy(out=topw5_sb[:NT, :], in_=ptw[:NT, :])

            # ---- LT-space segment means ---------------------------------
            acc_don = pp.tile([P, LLOC], F32, name="acc_don")
            acc_lob = pp.tile([P, LLOC], F32, name="acc_lob")
            acc_mem = pp.tile([P, LLOC], F32, name="acc_mem")
            nc.vector.memset(acc_don[:], 0.0)
            nc.vector.memset(acc_lob[:], 0.0)
            nc.vector.memset(acc_mem[:], 0.0)
            _emit_segsum(nc, tc, gpool, opool, pspool, rels["don"], acc_don,
                         "dmaj", t_hdon.ap(), *rel_sb["don"], iota_sb, "don")
            _emit_segsum(nc, tc, gpool, opool, pspool, rels["lob"], acc_lob,
                         "dmaj", t_hlob.ap(), *rel_sb["lob"], iota_sb, "lob")
            for w in range(NWL):
                cw = opool.tile([P, CH * P], F32, tag="cmem",
                                name=f"cmem_{w}")
                nc.sync.dma_start(
                    out=cw[:], in_=t_memC.ap()[:, w * CH * P:(w + 1) * CH * P])
                psm = pspool.tile([P, P], F32, tag="ps", name=f"psmem_{w}")
                for h in range(CH):
                    nc.tensor.matmul(out=psm[:], lhsT=hc[h][:],
                                     rhs=cw[:, h * P:(h + 1) * P],
                                     start=(h == 0), stop=(h == CH - 1))
                nc.vector.tensor_copy(out=acc_mem[:, w * P:(w + 1) * P],
                                      in_=psm[:])

            # LT table blocks -> ltb_dram, then AllGather
            for lb in range(NWL):
                sl = slice(lb * P, (lb + 1) * P)
                p0 = pspool.tile([P, P], F32, tag="ps", name=f"plt0_{lb}")
                nc.tensor.matmul(out=p0[:], lhsT=hltT_sb[:, sl],
                                 rhs=w_sb[0][:], start=True, stop=True)
                pm = pspool.tile([P, P], F32, tag="ps", name=f"pltm_{lb}")
                nc.tensor.matmul(out=pm[:], lhsT=acc_mem[:, sl],
                                 rhs=w_sb[3][:], start=True, stop=True)
                pd = pspool.tile([P, P], F32, tag="ps", name=f"pltd_{lb}")
                nc.tensor.matmul(out=pd[:], lhsT=acc_don[:, sl],
                                 rhs=w_sb[4][:], start=True, stop=True)
                sd = spool.tile([P, P], F32, tag="t", name=f"sltd_{lb}")
                nc.scalar.activation(out=sd[:], in_=pd[:], func=Copy,
                                     scale=rdon_sb[:, lb:lb + 1])
                pl = pspool.tile([P, P], F32, tag="ps", name=f"pltl_{lb}")
                nc.tensor.matmul(out=pl[:], lhsT=acc_lob[:, sl],
                                 rhs=w_sb[4][:], start=True, stop=True)
                sl2 = spool.tile([P, P], F32, tag="t", name=f"sltl_{lb}")
                nc.scalar.activation(out=sl2[:], in_=pl[:], func=Copy,
                                     scale=rlob_sb[:, lb:lb + 1])
                tt = spool.tile([P, P], F32, tag="t2", name=f"tlt_{lb}")
                nc.vector.tensor_add(out=tt[:], in0=p0[:], in1=sd[:])
                nc.vector.tensor_add(out=tt[:], in0=tt[:], in1=sl2[:])
                nc.vector.tensor_add(out=tt[:], in0=tt[:], in1=pm[:])
                nc.vector.tensor_add(out=tt[:], in0=tt[:], in1=bias_sb[:])
                nc.sync.dma_start(out=ltb_dram[lb * P:(lb + 1) * P, :],
                                  in_=tt[:])
            nc.gpsimd.collective_compute(
                "AllGather", mybir.AluOpType.bypass,
                replica_groups=[list(range(CORES))],
                ins=[ltb_dram.opt()], outs=[ltfull_dram.opt()])

            # ---- rd segment means (v-major) + bill table ----------------
            acc_rd, free_rd = tc.tile([P, NWV * P], F32, name="acc_rd")
            for w in range(NWV):
                cw = opool.tile([P, CH * P], F32, tag="cmem",
                                name=f"crd_{w}")
                nc.sync.dma_start(
                    out=cw[:], in_=t_rdC.ap()[:, w * CH * P:(w + 1) * CH * P])
                psr = pspool.tile([P, P], F32, tag="ps", name=f"psrd_{w}")
                for h in range(CH):
                    nc.tensor.matmul(out=psr[:],
                                     lhsT=cw[:, h * P:(h + 1) * P],
                                     rhs=hc[h][:],
                                     start=(h == 0), stop=(h == CH - 1))
                nc.vector.tensor_copy(out=acc_rd[:, w * P:(w + 1) * P],
                                      in_=psr[:])
            for bb in range(nbb):
                pbc = pspool.tile([P, P], F32, tag="ps", name=f"pbc_{bb}")
                for i in range(NVB):
                    col = bb * NVB + i
                    ob = opool.tile([P, P], F32, tag="ob", name=f"ob_{bb}_{i}")
                    nc.vector.tensor_tensor(
                        out=ob[:], in0=iota_sb[:, :P],
                        in1=billloc_sb[:, col:col + 1].to_broadcast([P, P]),
                        op=mybir.AluOpType.is_equal)
                    vb = bb * NVB + i
                    nc.tensor.matmul(
                        out=pbc[:], lhsT=acc_rd[:, vb * P:(vb + 1) * P],
                        rhs=ob[:], start=(i == 0), stop=(i == NVB - 1))
                bc = spool.tile([P, P], F32, tag="t", name=f"bc_{bb}")
                nc.vector.tensor_copy(out=bc[:], in_=pbc[:])
                pbt = pspool.tile([P, P], F32, tag="ps", name=f"pbt_{bb}")
                nc.tensor.matmul(out=pbt[:], lhsT=bc[:], rhs=w_sb[2][:],
                                 start=True, stop=True)
                bt = spool.tile([P, P], F32, tag="t2", name=f"bt_{bb}")
                nc.scalar.activation(out=bt[:], in_=pbt[:], func=Copy,
                                     scale=rnv_sb[:, bb:bb + 1])
                # topic addend via one-hot + transpose (no gather)
                otx = opool.tile([P, P], F32, tag="ob", name=f"otx_{bb}")
                nc.vector.tensor_tensor(
                    out=otx[:], in0=iota_sb[:, :P],
                    in1=tixf_sb[:, bb:bb + 1].to_broadcast([P, P]),
                    op=mybir.AluOpType.is_equal)
                ptx = pspool.tile([P, P], F32, tag="ps", name=f"ptx_{bb}")
                nc.tensor.transpose(out=ptx[:], in_=otx[:],
                                    identity=ident_sb[:])
                otxt = spool.tile([P, P], F32, tag="t", name=f"otxt_{bb}")
                nc.vector.tensor_copy(out=otxt[:], in_=ptx[:])
                ptp = pspool.tile([P, P], F32, tag="ps", name=f"ptp_{bb}")
                nc.tensor.matmul(out=ptp[:], lhsT=otxt[:], rhs=topw5_sb[:],
                                 start=True, stop=True)
                nc.vector.tensor_add(out=bill_sb[:, bb * D:(bb + 1) * D],
                                     in0=bt[:], in1=ptp[:])

            # ---- pv segment means + BV table ----------------------------
            free_rd()
            acc_pv, free_pv = tc.tile([P, VLOC], F32, name="acc_pv")
            nc.vector.memset(acc_pv[:], 0.0)
            _emit_segsum(nc, tc, gpool, opool, pspool, rels["pv"], acc_pv,
                         "dmaj", t_hbv.ap(), *rel_sb["pv"], iota_sb, "pv")
            for vb in range(NWV):
                ppv = pspool.tile([P, P], F32, tag="ps", name=f"ppv_{vb}")
                nc.tensor.matmul(out=ppv[:],
                                 lhsT=acc_pv[:, vb * P:(vb + 1) * P],
                                 rhs=w_sb[1][:], start=True, stop=True)
                sv = spool.tile([P, P], F32, tag="t", name=f"sv_{vb}")
                nc.scalar.activation(out=sv[:], in_=ppv[:], func=Copy,
                                     scale=rpv_sb[:, vb:vb + 1])
                ovb = opool.tile([P, P], F32, tag="ob", name=f"ovb_{vb}")
                nc.vector.tensor_tensor(
                    out=ovb[:], in0=iota_sb[:, :P],
                    in1=billloc_sb[:, vb:vb + 1].to_broadcast([P, P]),
                    op=mybir.AluOpType.is_equal)
                pvb = pspool.tile([P, P], F32, tag="ps", name=f"pvb_{vb}")
                nc.tensor.transpose(out=pvb[:], in_=ovb[:],
                                    identity=ident_sb[:])
                ovbt = spool.tile([P, P], F32, tag="t2", name=f"ovbt_{vb}")
                nc.vector.tensor_copy(out=ovbt[:], in_=pvb[:])
                bb = vb // NVB
                pba = pspool.tile([P, P], F32, tag="ps", name=f"pba_{vb}")
                nc.tensor.matmul(out=pba[:], lhsT=ovbt[:],
                                 rhs=bill_sb[:, bb * D:(bb + 1) * D],
                                 start=True, stop=True)
                nc.vector.tensor_add(out=sv[:], in0=sv[:], in1=pba[:])
                nc.sync.dma_start(out=bv_dram[vb * P:(vb + 1) * P, :],
                                  in_=sv[:])
            free_pv()

            # ---- final edge pass ----------------------------------------
            for su in range(ESLOT // SUP):
                glt = gpool.tile([P, SUP * D], F32, tag="g",
                                 name=f"glt_{su}")
                gbv = gpool.tile([P, SUP * D], F32, tag="g",
                                 name=f"gbv_{su}")
                for j in range(SUP):
                    s = su * SUP + j
                    nc.gpsimd.indirect_dma_start(
                        out=glt[:, j * D:(j + 1) * D], out_offset=None,
                        in_=ltfull_dram[:],
                        in_offset=bass.IndirectOffsetOnAxis(
                            ap=vlt_sb[:, s:s + 1], axis=0))
                    nc.gpsimd.indirect_dma_start(
                        out=gbv[:, j * D:(j + 1) * D], out_offset=None,
                        in_=bv_dram[:],
                        in_offset=bass.IndirectOffsetOnAxis(
                            ap=vbv_sb[:, s:s + 1], axis=0))
                nc.vector.tensor_add(out=glt[:], in0=glt[:], in1=gbv[:])
                nc.sync.dma_start(
                    out=t_out.ap()[su * SUP * P:(su + 1) * SUP * P, :]
                    .rearrange("(g p) d -> p g d", p=P),
                    in_=glt[:].rearrange("p (g d) -> p g d", d=D))

            if debug:
                nc.sync.dma_start(out=t_dbg["ltfull"].ap(),
                                  in_=ltfull_dram[:])
                nc.sync.dma_start(out=t_dbg["bv"].ap(), in_=bv_dram[:])
                nc.sync.dma_start(out=t_dbg["accdon"].ap(), in_=acc_don[:])

    nc.compile()
    return nc


# ---------------------------------------------------------------------------
# entry point
# ---------------------------------------------------------------------------

def kernel(**inputs):
    global _LAST_EXEC_NS
    plan, in_maps = _prep(inputs)
    nc = _build(plan)

    from concourse import bass_utils
    trace = os.environ.get("BASSK_TRACE", "0") == "1"
    if trace:
        try:
            import ntff_shim  # noqa: F401
        except ImportError:
            pass
    res = bass_utils.run_bass_kernel_spmd(
        nc, in_maps, core_ids=list(range(CORES)), trace=trace)
    _LAST_EXEC_NS = res.exec_time_ns

    E = plan["E"]
    out = np.zeros((E, D), np.float32)
    for c in range(CORES):
        ids = plan["core_orig"][c]
        out[ids] = res.results[c]["out"][:len(ids)]
    return out
